# revision 49
# baseline (speedup 1.0000x reference)
"""Bass/Trainium2 kernel for nn_EnhancedBilinearInteraction.

Reference computation:
    xp = W2 @ (W1 @ x[b,l,:] + b1) + b2      (per token, -> [B, 512, L])
    yp = same for y
    out[b,h] = sum_l tanh(xp[b,h,l]) * tanh(yp[b,h,l])

Host-side rewrite: W_eff = W2 @ W1 ([512, 256]), b_eff = W2 @ b1 + b2.

Engine plan (per core):
  PE   - fp8e4m3 DoubleRow matmuls (0.5 cyc/row, 2x fp16 throughput,
         256-deep contraction per instruction): an error-compensated
         3-matmul scheme psum = Ws8@x8 + (Ws8/16)@xlo8 + (dWs8/16)@x8
         where xlo8 = fp8((x - x8)*16) and dWs8 = fp8((W - W8)*1024).
         Matmul out capped at 512 fp32/partition (one PSUM bank, s3d3
         ISA rule walrus enforces but CoreSim/TimelineSim do not).
  ACT  - all 32 tanh tiles: tanh(psum/64 + b_eff) straight from PSUM,
         fp16 out; the bottleneck engine, gapless from first psum to the
         final tile. Every psum consumer occupies one of the two 4-bank
         PSUM slots for its full read, and nothing reads psum faster than
         ACT's 1 elem/lane-cycle, so offloading tanh tiles to DVE (deg-7
         poly) or Pool (extract) makes the 2-slot rotation the bottleneck
         instead - measured slower. All-ACT is optimal here.
  DVE  - 16 fused multiply+reduce products via the custom-ucode
         TENSOR_TENSOR_REDUCE op (the raw ISA opcode and Pool-engine
         TensorScalarPtr both fail walrus's opcode-on-engine check; the
         custom op has no fp16 perf mode, 2194ns/tile).
  DMA  - one descriptor per (tensor, chunk): the HWDGE descriptor pipe
         is serial at ~625ns/descriptor, so merged loads beat per-k
         slices; first-project deps y0, w, dw, bias ordered first and the
         startup tile skips its xlo compensation so yl0 (the last
         first-chunk transfer) gates nothing.

Timeline (TimelineSim): first ACT 5.9us (startup floor: serial DMA chain
+900ns DMA-sem), ACT span 60.9us (32 gapless tiles), tail 4.2us (last
TTR half + HWDGE/DGE fixed costs + DMA sem + end barriers) = 71.0us.

Error budget (threshold 2e-2): 1.28e-3 from the fp8 scheme everywhere
except the startup tile's cells (1/16 of outputs) at 1.11e-2 from its
dropped x-residual term. Inputs are deterministic (fixed seed), and the
numpy emulation of this pipeline matches device results within 3%.

Sharding: pure data parallel - batch dim (32) split across 8 cores.
"""

import numpy as np

B, L, C, H = 32, 2048, 256, 512
NCORES = 8
BPC = B // NCORES          # 4 batches per core
TOK = BPC * L              # 8192 tokens per core
CHUNK = L                  # one batch per chunk -> reduce maps 1:1 to out col
KT = C // 128              # 2 contraction tiles (= the DoubleRow pair dim)
MT = H // 128              # 4 output-row tiles
NCH = TOK // CHUNK         # 4 chunks per core

WS = 64.0                  # W stored as W*64 in fp8; undone by ACT/DVE scale
LS = 16.0                  # x residual stored as (x - x8)*16

_CACHE = {}


def _build():
    import concourse.tile as tile
    from concourse import bacc, mybir

    F8 = mybir.dt.float8e4
    F16 = mybir.dt.float16
    F32 = mybir.dt.float32
    Alu = mybir.AluOpType
    Tanh = mybir.ActivationFunctionType.Tanh
    DR = mybir.MatmulPerfMode.DoubleRow

    nc = bacc.Bacc(
        "TRN2", target_bir_lowering=False, debug=False, num_devices=NCORES
    )
    x8d = nc.dram_tensor("x8", [C, TOK], F8, kind="ExternalInput").ap()
    xl8d = nc.dram_tensor("xl8", [C, TOK], F8, kind="ExternalInput").ap()
    y8d = nc.dram_tensor("y8", [C, TOK], F8, kind="ExternalInput").ap()
    yl8d = nc.dram_tensor("yl8", [C, TOK], F8, kind="ExternalInput").ap()
    wd = nc.dram_tensor("w8", [C, H], F8, kind="ExternalInput").ap()
    dwd = nc.dram_tensor("dw8_16", [C, H], F8, kind="ExternalInput").ap()
    bE = nc.dram_tensor("bE", [H], F32, kind="ExternalInput").ap()
    out = nc.dram_tensor("out", [BPC, H], F32, kind="ExternalOutput").ap()
    # The very last tile's reduction ships as two independent partials (no
    # s0 chain between its TTR halves - that handoff would delay the final
    # TTR); the host writes their sum into out[-1, -128:] during the
    # unshard gather.
    out2 = nc.dram_tensor("out2", [2, 128], F32, kind="ExternalOutput").ap()

    with tile.TileContext(nc) as tc:
        with (
            tc.tile_pool(name="singles", bufs=1) as singles,
            tc.tile_pool(name="acts", bufs=8) as apool,
            tc.tile_pool(name="prods", bufs=4) as ppool,
            tc.tile_pool(name="psum", bufs=2, space="PSUM") as pspool,
        ):
            wt = singles.tile([128, KT, H], F8)
            dwt = singles.tile([128, KT, H], F8)
            bsb = singles.tile([128, MT], F32)
            out_sb = singles.tile([128, MT, BPC], F32)
            out_fin = singles.tile([128, MT, BPC], F32)

            xt = singles.tile([128, KT, TOK], F8)
            xlt = singles.tile([128, KT, TOK], F8)
            yt = singles.tile([128, KT, TOK], F8)
            ylt = singles.tile([128, KT, TOK], F8)

            def load(dst, src, j, lo=0, hi=CHUNK):
                # One DMA per (tensor, chunk): the HWDGE descriptor pipe is
                # the serial resource (~625ns each), so merge the k-tiles.
                sl = slice(j * CHUNK + lo, j * CHUNK + hi)
                nc.sync.dma_start(
                    out=dst[:, :, sl],
                    in_=src.rearrange("(t p) m -> p t m", p=128)[:, :, sl],
                )

            # Weights first, then per-chunk inputs in emission order: each
            # chunk's y (projected first, feeding the DVE chain) before its x.
            # First-project deps in matmul order: y0, yl0, then the three
            # stationaries (mm1 needs only wt, mm2 wlt, mm3 dwt), then bias.
            # (Splitting y0/yl0 for an earlier partial start loses: the
            # serial ~625ns/descriptor HWDGE pipe charges per DMA, so the
            # extra descriptors push the full-tile deps out further than the
            # partial start saves.)
            # Bias before yl0: the first (2-matmul) tile's ACT needs it and
            # does not need yl0. Exactly four DMAs precede yl0: the HWDGE
            # pipe issues one DMA per ~625ns, so a fifth slot would make
            # yl0 HWDGE-bound (+1.3us). Splitting y0 into halves (with a
            # half-psum first tile and a 2mm m1 so yl0 gates nothing before
            # m2) was measured +119ns: the extra slot makes every small
            # leading transfer HWDGE-bound, y0b lands at 6.2us, and an
            # unfillable 592ns ACT bubble opens between the first-tile
            # halves. Slicing the stationaries is worse still (+1.8us).
            load(yt, y8d, 0)
            nc.sync.dma_start(out=wt, in_=wd.rearrange("(t p) m -> p t m", p=128))
            nc.sync.dma_start(out=dwt, in_=dwd.rearrange("(t p) m -> p t m", p=128))
            nc.sync.dma_start(out=bsb, in_=bE.rearrange("(m p) -> p m", p=128))
            load(ylt, yl8d, 0)
            load(xt, x8d, 0)
            load(xlt, xl8d, 0)
            for j in range(1, NCH):
                load(yt, y8d, j)
                load(ylt, yl8d, j)
                load(xt, x8d, j)
                load(xlt, xl8d, j)

            # PE warmup toward full clock while the first input DMA lands.
            junk = singles.tile([128, 128], F16)
            nc.vector.memset(junk, 0.0)
            # 44 iterations: enough to keep PE continuously busy until the
            # first real matmuls (~4.7us) so they run at full clock rather
            # than dropping back to the mid p-state.
            psjunk = pspool.tile([128, 96], F32, tag="ps")
            for _ in range(44):
                nc.tensor.matmul(psjunk, junk, junk[:, :96], start=True, stop=True)
            # Pull the ~1.3us tanh table load off the critical path.
            junk_act = singles.tile([128, 1], F16)
            nc.scalar.activation(junk_act, junk[:, :1], Tanh, bias=0.0)

            def project(src, srclo, j, m, lo_last=False, ps=None, qs=range(4),
                        no_lo=False):
                """ps = Ws8@src8 + (Ws8/16)@srclo8 + (dWs8/16)@src8, units W*64.

                Matmul out is capped at 512 fp32 per partition (one PSUM
                bank) by the s3d3 ISA check, so emit 4 groups per chunk.
                lo_last: accumulation is commutative, so the srclo-dependent
                matmuls go last - at kernel start the src8 data lands a DMA
                transfer earlier than srclo and PE can begin immediately.
                ps/qs allow emitting a subset of the 4 groups into an
                existing tile; no_lo drops the srclo compensation term
                entirely (2-matmul variant for the startup tile)."""
                if ps is None:
                    ps = pspool.tile([128, CHUNK], F32, tag="ps")
                mb = slice(m * 128, (m + 1) * 128)

                def sl_o(q):
                    sl = slice(j * CHUNK + q * 512, j * CHUNK + (q + 1) * 512)
                    return sl, ps[:, q * 512 : (q + 1) * 512]

                if no_lo:
                    # w pass fully first: dw lands one transfer later, and
                    # the 4-deep PE wait queue would otherwise block ready
                    # w matmuls behind dw-gated ones.
                    for q in qs:
                        sl, o = sl_o(q)
                        nc.tensor.matmul(o, wt[:, :, mb], src[:, :, sl],
                                         start=True, stop=False, perf_mode=DR)
                    for q in qs:
                        sl, o = sl_o(q)
                        nc.tensor.matmul(o, dwt[:, :, mb], src[:, :, sl],
                                         start=False, stop=True, perf_mode=DR)
                elif lo_last:
                    for q in qs:
                        sl, o = sl_o(q)
                        nc.tensor.matmul(o, wt[:, :, mb], src[:, :, sl],
                                         start=True, stop=False, perf_mode=DR)
                        nc.tensor.matmul(o, dwt[:, :, mb], src[:, :, sl],
                                         start=False, stop=False, perf_mode=DR)
                    for q in qs:
                        sl, o = sl_o(q)
                        nc.tensor.matmul(o, wt[:, :, mb], srclo[:, :, sl],
                                         start=False, stop=True, perf_mode=DR)
                else:
                    for q in qs:
                        sl, o = sl_o(q)
                        nc.tensor.matmul(o, wt[:, :, mb], src[:, :, sl],
                                         start=True, stop=False, perf_mode=DR)
                        nc.tensor.matmul(o, wt[:, :, mb], srclo[:, :, sl],
                                         start=False, stop=False, perf_mode=DR)
                        nc.tensor.matmul(o, dwt[:, :, mb], src[:, :, sl],
                                         start=False, stop=True, perf_mode=DR)
                return ps

            def act_tanh(ps, m, tag):
                t = apool.tile([128, CHUNK], F16, tag=tag)
                nc.scalar.activation(
                    t, ps, Tanh, bias=bsb[:, m : m + 1], scale=1.0 / WS
                )
                return t

            from concourse.dve_ops import TENSOR_TENSOR_REDUCE

            def dve_product(xa, ya, m, j):
                prod = ppool.tile([128, CHUNK], F16, tag="prod")
                nc.vector._custom_dve(
                    TENSOR_TENSOR_REDUCE, out=prod, in0=xa, in1=ya,
                    s0=0.0, s1=1.0, accum_out=out_sb[:, m, j : j + 1],
                )

            # Chunk 0: y DMA lands first, so do all y-side tanh before x.
            # The very first tile (m=0, y) skips the x-residual compensation
            # matmuls: its psum then depends only on y0/w/dw and not on yl0
            # (the last first-chunk DMA transfer), starting ACT ~1.4us
            # earlier. Costs ~1.1e-2 rel err on 1/16 of the output cells
            # (measured vs the 2e-2 budget; the error does not stack with
            # other cells' contributions - max, not sum).
            yas = [act_tanh(project(yt, ylt, 0, 0, no_lo=True), 0, "ya")]
            for m in range(1, MT):
                yas.append(act_tanh(project(yt, ylt, 0, m, lo_last=True), m, "ya"))
            for m in range(MT):
                xa = act_tanh(project(xt, xlt, 0, m, lo_last=True), m, "xa")
                dve_product(xa, yas[m], m, 0)
            for j in range(1, NCH):
                if j == NCH - 1:
                    # The final row's first batches are complete after chunk
                    # 2: ship them now so only batch 3 trails the last TTR.
                    mf = MT - 1
                    nc.vector.tensor_copy(
                        out_fin[:, mf, : BPC - 1], out_sb[:, mf, : BPC - 1]
                    )
                    nc.sync.dma_start(
                        out=out[: BPC - 1, mf * 128 :].rearrange("b p -> p b"),
                        in_=out_fin[:, mf, : BPC - 1],
                    )
                for m in range(MT):
                    xa = act_tanh(project(xt, xlt, j, m), m, "xa")
                    if j == NCH - 1 and m == MT - 1:
                        # Final tile: activation + product in halves so the
                        # last TTR overlaps the last ACT; the second half
                        # accumulates straight into the ship buffer so the
                        # output DMA needs no copy. Each half is projected
                        # into its OWN half-width psum tile: a shared tile
                        # would serialize the two ACT reads (+219ns accessor
                        # chain) and delay the first read behind a 12-matmul
                        # chain that can only start when the previous slot
                        # frees; 6-matmul half tiles start earlier and the
                        # reads decouple. 1024+1024 is the optimum - the
                        # 222ns ACT->TTR sem ties the 134ns TTR handoff.
                        acc = out_sb[:, m, j : j + 1]
                        bias = bsb[:, m : m + 1]
                        mb = slice(m * 128, (m + 1) * 128)
                        prod = ppool.tile([128, CHUNK], F16, tag="prod")
                        ps_h = []
                        for half in range(2):
                            psh = pspool.tile([128, CHUNK // 2], F32, tag="ps")
                            for qi, q in enumerate((2 * half, 2 * half + 1)):
                                sl = slice(j * CHUNK + q * 512,
                                           j * CHUNK + (q + 1) * 512)
                                o = psh[:, qi * 512 : (qi + 1) * 512]
                                nc.tensor.matmul(o, wt[:, :, mb], yt[:, :, sl],
                                                 start=True, stop=False,
                                                 perf_mode=DR)
                                nc.tensor.matmul(o, wt[:, :, mb], ylt[:, :, sl],
                                                 start=False, stop=False,
                                                 perf_mode=DR)
                                nc.tensor.matmul(o, dwt[:, :, mb], yt[:, :, sl],
                                                 start=False, stop=True,
                                                 perf_mode=DR)
                            ps_h.append(psh)
                        fin2 = singles.tile([128, 2], F32)
                        for i in range(2):
                            a, b = i * 1024, (i + 1) * 1024
                            yap = apool.tile([128, 1024], F16, tag=f"yaf{i}")
                            nc.scalar.activation(
                                yap, ps_h[i], Tanh,
                                bias=bias, scale=1.0 / WS,
                            )
                            # Independent partials (s0=0 for both): the host
                            # adds them, so the second TTR needs no handoff
                            # of the first one's accumulator value. One DMA
                            # ships both cells - each extra output DMA costs
                            # a serial 50ns sem-wait in the exit cascade.
                            nc.vector._custom_dve(
                                TENSOR_TENSOR_REDUCE, out=prod[:, a:b],
                                in0=xa[:, a:b], in1=yap,
                                s0=0.0, s1=1.0,
                                accum_out=fin2[:, i : i + 1],
                            )
                        nc.sync.dma_start(
                            out=out2.rearrange("b p -> p b"), in_=fin2
                        )
                    else:
                        ya = act_tanh(project(yt, ylt, j, m), m, "ya")
                        dve_product(xa, ya, m, j)
                    if j == NCH - 1 and m != MT - 1:
                        # Funnel this m's accumulated row through a DVE copy
                        # (standard first-output write) and ship it at once;
                        # m<3 rows finish well before the final tile.
                        nc.vector.tensor_copy(out_fin[:, m, :], out_sb[:, m, :])
                        nc.sync.dma_start(
                            out=out[:, m * 128 : (m + 1) * 128].rearrange("b p -> p b"),
                            in_=out_fin[:, m, :],
                        )
    nc.compile()
    return nc


def _prep_inputs(x, y, W1, b1, W2, b2):
    import ml_dtypes

    F8NP = ml_dtypes.float8_e4m3
    x, y, W1, b1, W2, b2 = (
        np.asarray(t, dtype=np.float32) for t in (x, y, W1, b1, W2, b2)
    )
    W = (W2.astype(np.float64) @ W1.astype(np.float64)).astype(np.float32)  # [H, C]
    b_eff = (W2.astype(np.float64) @ b1.astype(np.float64) + b2).astype(np.float32)

    def q8(a):
        return a.astype(F8NP)

    WT = np.ascontiguousarray(W.T)                    # [C, H]
    ws8 = q8(WT * WS)                                 # values W*64
    dW = WT - ws8.astype(np.float32) / WS
    dws8_16 = q8(q8(dW * WS * LS).astype(np.float32) / LS)  # values dW*64

    in_maps = []
    for i in range(NCORES):
        xs = x[i * BPC : (i + 1) * BPC].reshape(TOK, C).T  # [C, TOK]
        ys = y[i * BPC : (i + 1) * BPC].reshape(TOK, C).T
        xs8 = q8(np.ascontiguousarray(xs))
        ys8 = q8(np.ascontiguousarray(ys))
        # residuals stored unscaled: their small values lose relative fp8
        # precision only below the subnormal floor (~2^-13 absolute), which
        # is negligible vs the W-quantization term; this lets the residual
        # matmuls share the main Ws8 stationary (one less weight DMA).
        xl8 = q8(xs - xs8.astype(np.float32))
        yl8 = q8(ys - ys8.astype(np.float32))
        in_maps.append(
            {
                "x8": xs8, "xl8": xl8, "y8": ys8, "yl8": yl8,
                "w8": ws8, "dw8_16": dws8_16,
                "bE": b_eff,
            }
        )
    return in_maps


def _run(inputs, trace=False):
    from concourse.bass_utils import run_bass_kernel_spmd

    if "nc" not in _CACHE:
        _CACHE["nc"] = _build()
    nc = _CACHE["nc"]
    in_maps = _prep_inputs(**inputs)
    # Retry on transient device wedges (NRT_EXEC_UNIT_UNRECOVERABLE).
    import time

    last_exc = None
    for attempt in range(3):
        try:
            res = run_bass_kernel_spmd(
                nc, in_maps, core_ids=list(range(NCORES)), trace=trace
            )
            break
        except Exception as e:  # noqa: BLE001
            last_exc = e
            time.sleep(5 * (attempt + 1))
    else:
        raise last_exc
    outs = []
    for r in res.results:
        o = np.array(r["out"])  # [BPC, H]
        # Final tile's reduction was shipped as two independent partials.
        o[BPC - 1, (MT - 1) * 128 :] = r["out2"][0] + r["out2"][1]
        outs.append(o)
    full = np.concatenate(outs, axis=0)  # [B, H]
    return full, res


def kernel(x, y, W1, b1, W2, b2):
    full, _ = _run(dict(x=x, y=y, W1=W1, b1=b1, W2=W2, b2=b2))
    return full

